# revision 1
# baseline (speedup 1.0000x reference)
"""NeighborRoutingConv (GAT-style multi-head edge-softmax message passing) on 8 trn2 cores.

Strategy (v3, dma_gather edition):
  - Host folds attn into the weight matrix: a[n,k] = sum_i h[n,i]*c[k,i] with
    c[k,:] = sum_j attn[k,j] * W[k*32+j, :].  One matmul per node tile emits
    whaug[n] = [ (h @ W.T)(256) ; a(8) ; pad(to 320) ].
  - Phase 1 (replicated on every core): compute whaug for all N nodes into
    core-local DRAM (320 f32 row stride for dma_gather's 256B-granularity).
  - Phase 2 (dst-sharded): edges grouped by 128-node destination blocks;
    blocks bin-packed into (core, slot) pairs so per-slot chunk counts are
    compile-time constants shared by all cores (SPMD).  Edges of a block are
    split by src < HALF into segment A/B (dma_gather idx is int16).  Each
    segment ends with a "header" chunk whose 128 entries gather the block's
    own 128 dst rows (the segment matching the block's half; the other
    segment's header gathers row 0 junk).  Per block:
      * dma_gather whaug[src] rows (320 f32) per segment -> M0 [128, nch, 320]
      * a_dst[128,8] = hdrA.aux*wA + hdrB.aux*wB  (host-provided 0/1 weights)
      * sel[e, ci, d] = (dcol[e,ci]==d)  batched is_equal (one-hot masks)
      * per chunk: PE-transpose sel_ci -> S (d-major); a_dst_e = S.T @ a_dst
      * e_exp = exp(leakyrelu(a_src + a_dst_e)) batched, into M0 aux cols
      * msgs *= bcast(e_exp); per chunk one PE matmul accumulates
        [segment_sum(msgs) ; segment_sum(e_exp)] into PSUM [128, 264]
      * out_block = psum[:, :256] * bcast(1/(e_sum+eps)) -> DMA out.
  Softmax max-subtraction is skipped (mathematically identical; |a| <~ 10 so
  e_exp stays in fp32 range).
"""

import math
from contextlib import ExitStack

import numpy as np

P = 128
IN_DIM = 256
OUT_DIM = 256
K = 8
DK = 32
ROW = 320  # whaug row stride (f32): Wh(256) | a(8) | pad
AUX = OUT_DIM  # aux column offset
RHS = OUT_DIM + K  # 264 — matmul rhs width (msgs + e_exp)
NEG_SLOPE = 0.2
N_CORES = 8
SUPER = 4  # node tiles per phase-1 iteration (512 nodes)


def _ceil_div(a, b):
    return (a + b - 1) // b


def _wrap16(lst):
    """dma_gather idx layout: [128, len//16] int16; idx i at [i%16, i//16],
    replicated across the 8 groups of 16 partitions."""
    n = len(lst)
    assert n % 16 == 0
    base = np.asarray(lst, dtype=np.int16).reshape(n // 16, 16).T  # [16, cols]
    return np.tile(base, (8, 1))  # [128, cols]


def build_plan(edge_src, edge_dst, n_nodes, n_cores):
    n_pad = _ceil_div(n_nodes, P * SUPER) * P * SUPER
    HALF = n_pad // 2
    B = _ceil_div(n_nodes, P)
    J = _ceil_div(B, n_cores)

    perm = np.argsort(edge_dst, kind="stable")
    dsts = edge_dst[perm].astype(np.int64)
    srcs = edge_src[perm].astype(np.int64)
    bounds = np.searchsorted(dsts, np.arange(B + 1) * P)

    # per-block A/B edge lists
    blkA, blkB = [], []
    for b in range(B):
        lo, hi = int(bounds[b]), int(bounds[b + 1])
        s, d = srcs[lo:hi], dsts[lo:hi]
        am = s < HALF
        blkA.append((s[am], d[am]))
        blkB.append((s[~am], d[~am]))

    chunksA = np.array([_ceil_div(len(blkA[b][0]), P) + 1 for b in range(B)])
    chunksB = np.array([_ceil_div(len(blkB[b][0]), P) + 1 for b in range(B)])
    order = np.argsort(-(chunksA + chunksB), kind="stable")

    CPBA, CPBB = [], []
    assign = -np.ones((n_cores, J), dtype=np.int64)
    for j in range(J):
        grp = order[j * n_cores : (j + 1) * n_cores]
        CPBA.append(int(chunksA[grp].max()))
        CPBB.append(int(chunksB[grp].max()))
        for c, b in enumerate(grp):
            assign[c, j] = b
    NCH = [a + b for a, b in zip(CPBA, CPBB)]
    TOTCH = int(sum(NCH))
    TA = int(sum(CPBA))
    TB = int(sum(CPBB))

    gA = np.zeros((n_cores, P, TA * 8), dtype=np.int16)
    gB = np.zeros((n_cores, P, TB * 8), dtype=np.int16)
    dcol = -np.ones((n_cores, P, TOTCH), dtype=np.float32)
    wab = np.zeros((n_cores, P, 2 * J), dtype=np.float32)

    for c in range(n_cores):
        cbA = cbB = cbN = 0
        for j in range(J):
            na, nb = CPBA[j], CPBB[j]
            b = assign[c, j]
            listA = np.zeros(na * P, dtype=np.int64)
            listB = np.zeros(nb * P, dtype=np.int64)
            if b >= 0:
                base = b * P
                sA, dA = blkA[b]
                sB, dB = blkB[b]
                listA[: len(sA)] = sA
                listB[: len(sB)] = sB - HALF
                inA = base < HALF
                hdr = np.arange(P) + (base - (0 if inA else HALF))
                if inA:
                    listA[(na - 1) * P :] = hdr
                    wab[c, :, 2 * j] = 1.0
                else:
                    listB[(nb - 1) * P :] = hdr
                    wab[c, :, 2 * j + 1] = 1.0
                # dcol for real edges (segment A then B), slot i -> [i%128, i//128]
                for lst_d, off in ((dA, 0), (dB, na)):
                    n = len(lst_d)
                    if n:
                        s_ = np.arange(n)
                        dcol[c, s_ & (P - 1), cbN + off + (s_ >> 7)] = (
                            lst_d - base
                        ).astype(np.float32)
            gA[c, :, cbA * 8 : (cbA + na) * 8] = _wrap16(listA)
            gB[c, :, cbB * 8 : (cbB + nb) * 8] = _wrap16(listB)
            cbA += na
            cbB += nb
            cbN += na + nb

    return {
        "n_pad": n_pad,
        "HALF": HALF,
        "B": B,
        "J": J,
        "CPBA": CPBA,
        "CPBB": CPBB,
        "NCH": NCH,
        "TOTCH": TOTCH,
        "TA": TA,
        "TB": TB,
        "CPBMAX": max(NCH),
        "assign": assign,
        "gA": gA,
        "gB": gB,
        "dcol": dcol,
        "wab": wab,
    }


def build_program(plan, n_cores, use_f32r=False):
    import concourse.bass as bass
    import concourse.tile as tile
    from concourse import bacc, mybir

    f32 = mybir.dt.float32
    i16 = mybir.dt.int16
    f32r = mybir.dt.float32r

    def mmcast(ap):
        return ap.bitcast(f32r) if use_f32r else ap

    n_pad = plan["n_pad"]
    HALF = plan["HALF"]
    J = plan["J"]
    CPBA, CPBB, NCH = plan["CPBA"], plan["CPBB"], plan["NCH"]
    TOTCH, TA, TB = plan["TOTCH"], plan["TA"], plan["TB"]
    cpbmax = plan["CPBMAX"]
    NT = n_pad // (P * SUPER)
    CG = IN_DIM // P

    nc = bacc.Bacc("TRN2", target_bir_lowering=False, debug=False,
                   num_devices=n_cores)

    hT = nc.dram_tensor("hT", [IN_DIM, n_pad], f32, kind="ExternalInput")
    waugT = nc.dram_tensor("waugT", [IN_DIM, RHS], f32, kind="ExternalInput")
    gA_d = nc.dram_tensor("gA", [P, TA * 8], i16, kind="ExternalInput")
    gB_d = nc.dram_tensor("gB", [P, TB * 8], i16, kind="ExternalInput")
    dcol_d = nc.dram_tensor("dcol", [P, TOTCH], f32, kind="ExternalInput")
    wab_d = nc.dram_tensor("wab", [P, 2 * J], f32, kind="ExternalInput")
    iota_d = nc.dram_tensor("iota", [P, P], f32, kind="ExternalInput")
    ident_d = nc.dram_tensor("ident", [P, P], f32, kind="ExternalInput")
    out_d = nc.dram_tensor("out", [J * P, OUT_DIM], f32, kind="ExternalOutput")
    whaug = nc.dram_tensor("whaug", [n_pad, ROW], f32)

    with tile.TileContext(nc) as tc, ExitStack() as ctx:
        consts = ctx.enter_context(tc.tile_pool(name="consts", bufs=1))
        ctx1 = ctx.enter_context(ExitStack())
        p1in = ctx1.enter_context(tc.tile_pool(name="p1in", bufs=3))
        p1ps = ctx1.enter_context(tc.tile_pool(name="p1ps", bufs=2, space="PSUM"))
        p1st = ctx1.enter_context(tc.tile_pool(name="p1st", bufs=3))

        waug_sb = consts.tile([P, CG, RHS], f32)
        nc.sync.dma_start(out=waug_sb[:],
                          in_=waugT.ap().rearrange("(g p) r -> p g r", p=P))
        iota_sb = consts.tile([P, P], f32)
        nc.sync.dma_start(out=iota_sb[:], in_=iota_d.ap())
        ident_sb = consts.tile([P, P], f32)
        nc.sync.dma_start(out=ident_sb[:], in_=ident_d.ap())
        gA_sb = consts.tile([P, TA * 8], i16)
        nc.sync.dma_start(out=gA_sb[:], in_=gA_d.ap())
        gB_sb = consts.tile([P, TB * 8], i16)
        nc.sync.dma_start(out=gB_sb[:], in_=gB_d.ap())
        dcol_sb = consts.tile([P, TOTCH], f32)
        nc.sync.dma_start(out=dcol_sb[:], in_=dcol_d.ap())
        wab_sb = consts.tile([P, 2 * J], f32)
        nc.sync.dma_start(out=wab_sb[:], in_=wab_d.ap())

        # ---- phase 1 ----
        hT_r = hT.ap().rearrange("(g p) n -> p g n", p=P)
        wh_r = whaug.ap().rearrange("(i t p) r -> i p t r", t=SUPER, p=P)
        for it in range(NT):
            ht = p1in.tile([P, CG, SUPER * P], f32)
            nc.sync.dma_start(
                out=ht[:], in_=hT_r[:, :, it * SUPER * P : (it + 1) * SUPER * P]
            )
            ps = p1ps.tile([P, SUPER, 512], f32)
            for t in range(SUPER):
                for g in range(CG):
                    nc.tensor.matmul(
                        out=ps[:, t, 0:RHS],
                        lhsT=mmcast(ht[:, g, t * P : (t + 1) * P]),
                        rhs=mmcast(waug_sb[:, g, :]),
                        start=(g == 0),
                        stop=(g == CG - 1),
                    )
            st = p1st.tile([P, SUPER, ROW], f32)
            nc.vector.memset(st[:, :, RHS:ROW], 0.0)
            nc.scalar.copy(out=st[:, :, 0:RHS], in_=ps[:, :, 0:RHS])
            nc.gpsimd.dma_start(out=wh_r[it], in_=st[:])

        ctx1.close()
        tc.strict_bb_all_engine_barrier()

        # ---- phase 2 ----
        m0p = ctx.enter_context(tc.tile_pool(name="m0p", bufs=2))
        selp = ctx.enter_context(tc.tile_pool(name="selp", bufs=2))
        sps = ctx.enter_context(tc.tile_pool(name="sps", bufs=3, space="PSUM"))
        ssb = ctx.enter_context(tc.tile_pool(name="ssb", bufs=3))
        adp = ctx.enter_context(tc.tile_pool(name="adp", bufs=2, space="PSUM"))
        accp = ctx.enter_context(tc.tile_pool(name="accp", bufs=2, space="PSUM"))
        scp = ctx.enter_context(tc.tile_pool(name="scp", bufs=2))
        outp = ctx.enter_context(tc.tile_pool(name="outp", bufs=2))
        smallp = ctx.enter_context(tc.tile_pool(name="smallp", bufs=4))

        tabA = whaug.ap()[0:HALF, :]
        tabB = whaug.ap()[HALF:n_pad, :]
        cbA = cbB = cbN = 0
        for j in range(J):
            na, nb, nch = CPBA[j], CPBB[j], NCH[j]
            m0t = m0p.tile([P, cpbmax, ROW], f32)
            GMAX = 8  # chunks per dma_gather call (<=1024 descriptors)
            for tab, nseg, cb, gsb, off in (
                (tabA, na, cbA, gA_sb, 0),
                (tabB, nb, cbB, gB_sb, na),
            ):
                for c0 in range(0, nseg, GMAX):
                    cn = min(GMAX, nseg - c0)
                    nc.gpsimd.dma_gather(
                        out_ap=m0t[:, off + c0 : off + c0 + cn, :],
                        in_ap=tab,
                        idxs_ap=gsb[:, (cb + c0) * 8 : (cb + c0 + cn) * 8],
                        num_idxs=cn * P,
                        num_idxs_reg=cn * P,
                        elem_size=ROW,
                        elem_step=ROW,
                    )
            # a_dst[128,8] = hdrA.aux*wA + hdrB.aux*wB
            ad_sb = smallp.tile([P, K], f32)
            t1 = smallp.tile([P, K], f32)
            nc.vector.tensor_scalar(
                out=t1[:], in0=m0t[:, na - 1, AUX : AUX + K],
                scalar1=wab_sb[:, 2 * j : 2 * j + 1], scalar2=None,
                op0=mybir.AluOpType.mult,
            )
            nc.vector.scalar_tensor_tensor(
                out=ad_sb[:], in0=m0t[:, nch - 1, AUX : AUX + K],
                scalar=wab_sb[:, 2 * j + 1 : 2 * j + 2],
                in1=t1[:], op0=mybir.AluOpType.mult, op1=mybir.AluOpType.add,
            )
            # batched one-hot masks
            sel = selp.tile([P, cpbmax, P], f32)
            iv = iota_sb[:]
            dview = dcol_sb[:, cbN : cbN + nch]
            nc.vector.tensor_tensor(
                out=sel[:, 0:nch, :],
                in0=bass.AP(tensor=iv.tensor, offset=iv.offset,
                            ap=[iv.ap[0], [0, nch], [1, P]]),
                in1=bass.AP(tensor=dview.tensor, offset=dview.offset,
                            ap=[dview.ap[0], [1, nch], [0, P]]),
                op=mybir.AluOpType.is_equal,
            )
            # per-chunk: S = sel_ci^T (PE), a_dst_e = S.T @ a_dst
            adst = adp.tile([P, cpbmax, K], f32)
            for ci in range(nch):
                s_ps = sps.tile([P, P], f32)
                nc.tensor.transpose(out=s_ps[:], in_=sel[:, ci, :],
                                    identity=ident_sb[:])
                s_sb = ssb.tile([P, P], f32)
                nc.scalar.copy(out=s_sb[:], in_=s_ps[:])
                nc.tensor.matmul(out=adst[:, ci, :], lhsT=s_sb[:], rhs=ad_sb[:],
                                 start=True, stop=True)
            # e_exp = exp(leaky(a_src + a_dst_e)) -> M0 aux
            aux = m0t[:, 0:nch, AUX : AUX + K]
            s_t = scp.tile([P, cpbmax, K], f32)
            nc.vector.tensor_tensor(out=s_t[:, 0:nch, :], in0=aux,
                                    in1=adst[:, 0:nch, :],
                                    op=mybir.AluOpType.add)
            lk = scp.tile([P, cpbmax, K], f32)
            nc.vector.scalar_tensor_tensor(
                out=lk[:, 0:nch, :], in0=s_t[:, 0:nch, :], scalar=NEG_SLOPE,
                in1=s_t[:, 0:nch, :],
                op0=mybir.AluOpType.mult, op1=mybir.AluOpType.max,
            )
            nc.scalar.activation(out=aux, in_=lk[:, 0:nch, :],
                                 func=mybir.ActivationFunctionType.Exp)
            # msgs *= bcast(e_exp)
            msg4 = m0t[:, 0:nch, 0:OUT_DIM].rearrange("p n (k d) -> p n k d", k=K)
            nc.vector.tensor_tensor(
                out=msg4, in0=msg4,
                in1=bass.AP(tensor=aux.tensor, offset=aux.offset,
                            ap=[aux.ap[0], [ROW, nch], [1, K], [0, DK]]),
                op=mybir.AluOpType.mult,
            )
            acc = accp.tile([P, RHS], f32)
            for ci in range(nch):
                nc.tensor.matmul(
                    out=acc[:],
                    lhsT=mmcast(sel[:, ci, :]),
                    rhs=mmcast(m0t[:, ci, 0:RHS]),
                    start=(ci == 0),
                    stop=(ci == nch - 1),
                )
            r = smallp.tile([P, K], f32)
            nc.vector.tensor_scalar(
                out=r[:], in0=acc[:, AUX : AUX + K], scalar1=1e-38, scalar2=None,
                op0=mybir.AluOpType.add,
            )
            nc.vector.reciprocal(out=r[:], in_=r[:])
            ot = outp.tile([P, OUT_DIM], f32)
            nc.vector.tensor_tensor(
                out=ot[:], in0=acc[:, 0:OUT_DIM],
                in1=r[:].to_broadcast([P, K, DK]),
                op=mybir.AluOpType.mult,
            )
            nc.sync.dma_start(out=out_d.ap()[j * P : (j + 1) * P, :], in_=ot[:])
            cbA += na
            cbB += nb
            cbN += nch

    nc.compile()
    return nc


def run(h, edge_src, edge_dst, W, attn, n_cores=N_CORES, trace=False,
        use_f32r=False):
    from concourse.bass_utils import run_bass_kernel_spmd

    n_nodes = h.shape[0]
    h = np.asarray(h, dtype=np.float32)
    W = np.asarray(W, dtype=np.float32)
    attn = np.asarray(attn, dtype=np.float32)
    edge_src = np.asarray(edge_src)
    edge_dst = np.asarray(edge_dst)

    plan = build_plan(edge_src, edge_dst, n_nodes, n_cores)
    n_pad = plan["n_pad"]
    hTd = np.zeros((IN_DIM, n_pad), dtype=np.float32)
    hTd[:, :n_nodes] = h.T
    c = (attn[:, :, None] * W.reshape(K, DK, IN_DIM)).sum(axis=1)
    waugT = np.concatenate([W.T, c.T], axis=1).astype(np.float32)
    iota = np.tile(np.arange(P, dtype=np.float32), (P, 1))
    ident = np.eye(P, dtype=np.float32)

    nc = build_program(plan, n_cores, use_f32r=use_f32r)

    in_maps = []
    for cix in range(n_cores):
        in_maps.append({
            "hT": hTd,
            "waugT": waugT,
            "gA": plan["gA"][cix],
            "gB": plan["gB"][cix],
            "dcol": plan["dcol"][cix],
            "wab": plan["wab"][cix],
            "iota": iota,
            "ident": ident,
        })
    try:
        res = run_bass_kernel_spmd(nc, in_maps, list(range(n_cores)), trace=trace)
    except Exception:
        if not trace:
            raise
        res = run_bass_kernel_spmd(nc, in_maps, list(range(n_cores)), trace=False)

    out_full = np.zeros((plan["B"] * P, OUT_DIM), dtype=np.float32)
    for cix in range(n_cores):
        o = res.results[cix]["out"]
        for j in range(plan["J"]):
            b = plan["assign"][cix, j]
            if b >= 0:
                out_full[b * P : (b + 1) * P] = o[j * P : (j + 1) * P]
    out = out_full[:n_nodes].reshape(n_nodes, K, DK)
    return out, res


def kernel(h, edge_src, edge_dst, W, attn):
    out, _ = run(h, edge_src, edge_dst, W, attn)
    return out



# revision 7
# speedup vs baseline: 3.1400x; 3.1400x over previous
"""NeighborRoutingConv (GAT-style multi-head edge-softmax message passing) on 8 trn2 cores.

Strategy (v8, 120-dst blocks + merged sel/a_dst rows, f32-view gathers):
  - Host folds attn into the weight matrix (c = attn-weighted row sums of W).
  - Phase 1 (replicated): for all nodes compute Wh (256 bf16) -> whW table
    (512 B rows) and a (8 bf16) -> aux table (256 B rows), both with 64-row
    zero prologues per half so idx 0 is a safe zero row.
  - Blocks hold 120 destinations so a single 256 B row packs
    [one-hot(120) | a_dst(8)] bf16: one gather delivers both the scatter
    matrix column and the dst attention logits.
  - sdT table [ZP + J*120, 128 bf16]: static one-hot part copied DRAM->DRAM
    from a host constant; a_dst part (cols 120:128) written by a reorder pass
    that gathers each core's blocks' dst a-vectors from aux (A/B halves with
    zero-junk rows, combined by add).
  - Main loop (dst-sharded blocks, SPMD bin-packed; all gathers use f32
    bitcast views to halve the modeled element count):
      * m0 <- whW[src] rows (128 f32 elems)   = messages
      * m1 <- aux[src] rows (64 elems)        = a_src
      * mS <- sdT[j*120+dcol] rows (64 elems) = [sel one-hot | a_dst]
      * e = exp(lrelu(a_src + a_dst)) -> pair-duplicated bf16 (Act)
      * msgs *= bcast(e) via (chunk,k)-flattened [.,2] APs (DVE 2x mode)
      * per chunk: acc[0:120,0:256] += sel^T msgs ; accE[0:120,0:16] +=
        sel^T e_pairs (junk lanes hit zero rows and vanish)
      * out = acc * bcast(1/e_sum) -> DMA out f32.
"""

from contextlib import ExitStack

import numpy as np

P = 128
BP = 120  # dst nodes per block (one-hot width; +8 a_dst cols = 128)
IN_DIM = 256
OUT_DIM = 256
K = 8
DK = 32
RHS = OUT_DIM + K  # 264 phase-1 matmul width (Wh | a)
NEG_SLOPE = 0.2
N_CORES = 8
SUPER = 4  # node tiles per phase-1 iteration (512 nodes)
GMAX = 8  # chunks per dma_gather call (<=1024 descriptors)
ZP = 64  # zero-prologue rows per half


def _ceil_div(a, b):
    return (a + b - 1) // b


def _wrap16(lst):
    """dma_gather idx layout: [128, len//16] int16; idx i at [i%16, i//16],
    replicated across the 8 groups of 16 partitions."""
    n = len(lst)
    assert n % 16 == 0
    base = np.asarray(lst, dtype=np.int16).reshape(n // 16, 16).T  # [16, cols]
    return np.tile(base, (8, 1))  # [128, cols]


def build_plan(edge_src, edge_dst, n_nodes, n_cores):
    n_pad = _ceil_div(n_nodes, P * SUPER) * P * SUPER
    HALF = n_pad // 2
    B = _ceil_div(n_nodes, BP)
    J = _ceil_div(B, n_cores)

    perm = np.argsort(edge_dst, kind="stable")
    dsts = edge_dst[perm].astype(np.int64)
    srcs = edge_src[perm].astype(np.int64)
    bounds = np.searchsorted(dsts, np.arange(B + 1) * BP)

    # per-block A/B edge lists (split by src half for int16 gather indices)
    blkA, blkB = [], []
    for b in range(B):
        lo, hi = int(bounds[b]), int(bounds[b + 1])
        s, d = srcs[lo:hi], dsts[lo:hi]
        am = s < HALF
        blkA.append((s[am], d[am]))
        blkB.append((s[~am], d[~am]))

    chunksA = np.array([_ceil_div(max(len(blkA[b][0]), 1), P) for b in range(B)])
    chunksB = np.array([_ceil_div(max(len(blkB[b][0]), 1), P) for b in range(B)])

    # bin-pack: lexsort by (A, B) chunk counts desc, deal groups of n_cores
    order = np.lexsort((-chunksB, -chunksA))
    CPBA, CPBB = [], []
    assign = -np.ones((n_cores, J), dtype=np.int64)
    for j in range(J):
        grp = order[j * n_cores : (j + 1) * n_cores]
        CPBA.append(int(chunksA[grp].max()))
        CPBB.append(int(chunksB[grp].max()))
        for c, b in enumerate(grp):
            assign[c, j] = b
    NCH = [a + b for a, b in zip(CPBA, CPBB)]
    TOTCH = int(sum(NCH))
    TA = int(sum(CPBA))
    TB = int(sum(CPBB))
    JPAD = _ceil_div(J, GMAX) * GMAX

    gA = np.zeros((n_cores, P, TA * 8), dtype=np.int16)
    gB = np.zeros((n_cores, P, TB * 8), dtype=np.int16)
    gS = np.zeros((n_cores, P, TOTCH * 8), dtype=np.int16)  # sdT idx
    gRA = np.zeros((n_cores, P, JPAD * 8), dtype=np.int16)  # reorder A
    gRB = np.zeros((n_cores, P, JPAD * 8), dtype=np.int16)  # reorder B

    for c in range(n_cores):
        cbA = cbB = cbN = 0
        for j in range(J):
            na, nb, nch = CPBA[j], CPBB[j], NCH[j]
            b = assign[c, j]
            listA = np.zeros(na * P, dtype=np.int64)
            listB = np.zeros(nb * P, dtype=np.int64)
            listS = np.zeros(nch * P, dtype=np.int64)
            if b >= 0:
                base = b * BP
                sA, dA = blkA[b]
                sB, dB = blkB[b]
                listA[: len(sA)] = ZP + sA
                listB[: len(sB)] = ZP + (sB - HALF)
                # reorder gather: block's own 120 dst rows from aux (per-idx
                # half split; pad lanes 120..127 stay 0 -> zero rows)
                hdr = np.arange(BP) + base
                ra = np.where(hdr < HALF, ZP + hdr, 0)
                rb = np.where(hdr >= HALF, ZP + hdr - HALF, 0)
                gRA[c, :, j * 8 : (j + 1) * 8] = _wrap16(
                    np.concatenate([ra, np.zeros(P - BP, dtype=np.int64)]))
                gRB[c, :, j * 8 : (j + 1) * 8] = _wrap16(
                    np.concatenate([rb, np.zeros(P - BP, dtype=np.int64)]))
                # per-edge sdT idx (segment A then B)
                for lst_d, off, ne in ((dA, 0, len(sA)), (dB, na * P, len(sB))):
                    if ne:
                        s_ = np.arange(ne)
                        listS[off + s_] = ZP + j * BP + (lst_d - base)
            gA[c, :, cbA * 8 : (cbA + na) * 8] = _wrap16(listA)
            gB[c, :, cbB * 8 : (cbB + nb) * 8] = _wrap16(listB)
            gS[c, :, cbN * 8 : (cbN + nch) * 8] = _wrap16(listS)
            cbA += na
            cbB += nb
            cbN += nch

    return {
        "n_pad": n_pad,
        "HALF": HALF,
        "B": B,
        "J": J,
        "JPAD": JPAD,
        "CPBA": CPBA,
        "CPBB": CPBB,
        "NCH": NCH,
        "TOTCH": TOTCH,
        "TA": TA,
        "TB": TB,
        "CPBMAX": max(NCH),
        "assign": assign,
        "gA": gA,
        "gB": gB,
        "gS": gS,
        "gRA": gRA,
        "gRB": gRB,
    }


def build_program(plan, n_cores):
    import concourse.bass as bass
    import concourse.tile as tile
    from concourse import bacc, mybir

    f32 = mybir.dt.float32
    bf16 = mybir.dt.bfloat16
    i16 = mybir.dt.int16

    n_pad = plan["n_pad"]
    HALF = plan["HALF"]
    J = plan["J"]
    JPAD = plan["JPAD"]
    CPBA, CPBB, NCH = plan["CPBA"], plan["CPBB"], plan["NCH"]
    TOTCH, TA, TB = plan["TOTCH"], plan["TA"], plan["TB"]
    cpbmax = plan["CPBMAX"]
    NT = n_pad // (P * SUPER)
    NT_HALF = HALF // (P * SUPER)
    CG = IN_DIM // P
    AUXW = 128  # aux/sdT row width (bf16): 256 B rows
    A32 = AUXW // 2  # 64 f32 per aux/sdT row
    W32 = OUT_DIM // 2  # 128 f32 per whW row

    nc = bacc.Bacc("TRN2", target_bir_lowering=False, debug=False,
                   num_devices=n_cores)

    hT = nc.dram_tensor("hT", [IN_DIM, n_pad], bf16, kind="ExternalInput")
    waugT = nc.dram_tensor("waugT", [IN_DIM, RHS], bf16, kind="ExternalInput")
    gA_d = nc.dram_tensor("gA", [P, TA * 8], i16, kind="ExternalInput")
    gB_d = nc.dram_tensor("gB", [P, TB * 8], i16, kind="ExternalInput")
    gS_d = nc.dram_tensor("gS", [P, TOTCH * 8], i16, kind="ExternalInput")
    gRA_d = nc.dram_tensor("gRA", [P, JPAD * 8], i16, kind="ExternalInput")
    gRB_d = nc.dram_tensor("gRB", [P, JPAD * 8], i16, kind="ExternalInput")
    soh_d = nc.dram_tensor("soh", [J * BP, AUXW], bf16, kind="ExternalInput")
    out_d = nc.dram_tensor("out", [J * BP, OUT_DIM], f32, kind="ExternalOutput")
    # device tables: [zero x64 | A-half | zero x64 | B-half]
    whW = nc.dram_tensor("whW", [n_pad + 2 * ZP, OUT_DIM], bf16)
    aux = nc.dram_tensor("aux", [n_pad + 2 * ZP, AUXW], bf16)
    sdT = nc.dram_tensor("sdT", [ZP + J * BP, AUXW], bf16)

    with tile.TileContext(nc) as tc, ExitStack() as ctx:
        consts = ctx.enter_context(tc.tile_pool(name="consts", bufs=1))
        ctx1 = ctx.enter_context(ExitStack())
        p1in = ctx1.enter_context(tc.tile_pool(name="p1in", bufs=6))
        p1ps = ctx1.enter_context(tc.tile_pool(name="p1ps", bufs=2, space="PSUM"))
        p1w = ctx1.enter_context(tc.tile_pool(name="p1w", bufs=4))
        p1w2 = ctx1.enter_context(tc.tile_pool(name="p1w2", bufs=4))
        p1a = ctx1.enter_context(tc.tile_pool(name="p1a", bufs=4))

        waug_sb = consts.tile([P, CG, RHS], bf16)
        nc.sync.dma_start(out=waug_sb[:],
                          in_=waugT.ap().rearrange("(g p) r -> p g r", p=P))
        gA_sb = consts.tile([P, TA * 8], i16)
        nc.scalar.dma_start(out=gA_sb[:], in_=gA_d.ap())
        gB_sb = consts.tile([P, TB * 8], i16)
        nc.scalar.dma_start(out=gB_sb[:], in_=gB_d.ap())
        gS_sb = consts.tile([P, TOTCH * 8], i16)
        nc.scalar.dma_start(out=gS_sb[:], in_=gS_d.ap())
        gRA_sb = consts.tile([P, JPAD * 8], i16)
        nc.scalar.dma_start(out=gRA_sb[:], in_=gRA_d.ap())
        gRB_sb = consts.tile([P, JPAD * 8], i16)
        nc.scalar.dma_start(out=gRB_sb[:], in_=gRB_d.ap())

        # zero prologues + static one-hot part of sdT (DRAM->DRAM)
        zt = consts.tile([P, OUT_DIM], bf16)
        nc.vector.memset(zt[:], 0.0)
        for half in range(2):
            r0 = half * (HALF + ZP)
            nc.scalar.dma_start(out=whW.ap()[r0 : r0 + ZP, :], in_=zt[0:ZP, :])
            nc.scalar.dma_start(out=aux.ap()[r0 : r0 + ZP, :],
                                in_=zt[0:ZP, 0:AUXW])
        nc.scalar.dma_start(out=sdT.ap()[0:ZP, :], in_=zt[0:ZP, 0:AUXW])
        nc.scalar.dma_start(
            out=sdT.ap()[ZP : ZP + J * BP, :].rearrange(
                "(j b) w -> b j w", b=BP),
            in_=soh_d.ap().rearrange("(j b) w -> b j w", b=BP))

        # ---- phase 1: whW + aux tables for all nodes ----
        hT_r = hT.ap().rearrange("(g p) n -> p g n", p=P)
        # group phase-1 iters (<=4, never straddling the half boundary) so the
        # tiny aux writes amortize the per-DMA floor
        groups = []
        it = 0
        while it < NT:
            n = min(4, NT - it, (NT_HALF - it) if it < NT_HALF else NT)
            groups.append((it, n))
            it += n
        for g0, gn in groups:
            stA4 = p1a.tile([P, 4 * SUPER, K], bf16)
            for gi in range(gn):
                it = g0 + gi
                r0 = it * SUPER * P + (ZP if it < NT_HALF else 2 * ZP)
                ht = p1in.tile([P, CG, SUPER * P], bf16)
                nc.sync.dma_start(
                    out=ht[:],
                    in_=hT_r[:, :, it * SUPER * P : (it + 1) * SUPER * P]
                )
                ps = p1ps.tile([P, SUPER, 512], f32)
                for t in range(SUPER):
                    for g in range(CG):
                        nc.tensor.matmul(
                            out=ps[:, t, 0:RHS],
                            lhsT=ht[:, g, t * P : (t + 1) * P],
                            rhs=waug_sb[:, g, :],
                            start=(g == 0),
                            stop=(g == CG - 1),
                        )
                stX = p1w.tile([P, 2, OUT_DIM], bf16)
                stY = p1w2.tile([P, 2, OUT_DIM], bf16)
                # separate tiles so the Act and DVE copies overlap; both gate
                # the PSUM release so each is half an iteration
                nc.scalar.copy(out=stX[:], in_=ps[:, 0:2, 0:OUT_DIM])
                nc.vector.tensor_copy(out=stY[:], in_=ps[:, 2:4, 0:OUT_DIM])
                nc.vector.tensor_copy(
                    out=stA4[:, gi * SUPER : (gi + 1) * SUPER, :],
                    in_=ps[:, :, OUT_DIM:RHS])
                whr = whW.ap()[r0 : r0 + SUPER * P, :].rearrange(
                    "(t p) r -> p t r", p=P)
                nc.gpsimd.dma_start(out=whr[:, 0:2, :], in_=stX[:])
                nc.gpsimd.dma_start(out=whr[:, 2:4, :], in_=stY[:])
            ra = g0 * SUPER * P + (ZP if g0 < NT_HALF else 2 * ZP)
            nc.scalar.dma_start(
                out=aux.ap()[ra : ra + gn * SUPER * P, 0:K].rearrange(
                    "(t p) k -> p t k", p=P),
                in_=stA4[:, 0 : gn * SUPER, :])

        ctx1.close()
        tc.strict_bb_all_engine_barrier()

        # ---- reorder: sdT[ZP + j*120 + d, 120:128] = a of node b*120+d ----
        ctx2 = ctx.enter_context(ExitStack())
        rp = ctx2.enter_context(tc.tile_pool(name="rp", bufs=1))
        auxA = aux.ap()[0 : ZP + HALF, :].bitcast(f32)
        auxB = aux.ap()[ZP + HALF : n_pad + 2 * ZP, :].bitcast(f32)
        rA = rp.tile([P, JPAD, A32], f32)
        rB = rp.tile([P, JPAD, A32], f32)
        for c0 in range(0, JPAD, GMAX):
            cn = min(GMAX, JPAD - c0)
            for tab, gsb, dst in ((auxA, gRA_sb, rA), (auxB, gRB_sb, rB)):
                nc.gpsimd.dma_gather(
                    out_ap=dst[:, c0 : c0 + cn, :],
                    in_ap=tab,
                    idxs_ap=gsb[:, c0 * 8 : (c0 + cn) * 8],
                    num_idxs=cn * P,
                    num_idxs_reg=cn * P,
                    elem_size=A32,
                    elem_step=A32,
                )
        rab = rp.tile([P, J, K], bf16)
        nc.vector.tensor_tensor(
            out=rab[:],
            in0=rA[:].bitcast(bf16)[:, 0:J, 0:K],
            in1=rB[:].bitcast(bf16)[:, 0:J, 0:K],
            op=mybir.AluOpType.add,
        )
        nc.sync.dma_start(
            out=sdT.ap()[ZP : ZP + J * BP, BP:AUXW].rearrange(
                "(j p) k -> p j k", p=BP),
            in_=rab[0:BP, :, :])
        ctx2.close()
        tc.strict_bb_all_engine_barrier()

        # ---- main loop ----
        m0p = ctx.enter_context(tc.tile_pool(name="m0p", bufs=3))
        m1p = ctx.enter_context(tc.tile_pool(name="m1p", bufs=3))
        mSp = ctx.enter_context(tc.tile_pool(name="mSp", bufs=3))
        scp = ctx.enter_context(tc.tile_pool(name="scp", bufs=3))
        accp = ctx.enter_context(tc.tile_pool(name="accp", bufs=3, space="PSUM"))
        accEp = ctx.enter_context(tc.tile_pool(name="accEp", bufs=3, space="PSUM"))
        outp = ctx.enter_context(tc.tile_pool(name="outp", bufs=3))
        smallp = ctx.enter_context(tc.tile_pool(name="smallp", bufs=4))

        whA = whW.ap()[0 : ZP + HALF, :].bitcast(f32)
        whB = whW.ap()[ZP + HALF : n_pad + 2 * ZP, :].bitcast(f32)
        sdTf = sdT.ap().bitcast(f32)

        cbA = cbB = cbN = 0
        for j in range(J):
            na, nb, nch = CPBA[j], CPBB[j], NCH[j]
            m0 = m0p.tile([P, cpbmax, OUT_DIM], bf16)
            m1 = m1p.tile([P, cpbmax, A32], f32)
            mS = mSp.tile([P, cpbmax, A32], f32)
            # dst-side gather first: [one-hot | a_dst] rows from sdT
            for c0 in range(0, nch, GMAX):
                cn = min(GMAX, nch - c0)
                nc.gpsimd.dma_gather(
                    out_ap=mS[:, c0 : c0 + cn, :],
                    in_ap=sdTf,
                    idxs_ap=gS_sb[:, (cbN + c0) * 8 : (cbN + c0 + cn) * 8],
                    num_idxs=cn * P, num_idxs_reg=cn * P,
                    elem_size=A32, elem_step=A32,
                )
            # src gathers: a_src rows then Wh rows, A then B segments
            for tab, taba, nseg, cb, gsb, off in (
                (whA, auxA, na, cbA, gA_sb, 0),
                (whB, auxB, nb, cbB, gB_sb, na),
            ):
                for c0 in range(0, nseg, GMAX):
                    cn = min(GMAX, nseg - c0)
                    idxs = gsb[:, (cb + c0) * 8 : (cb + c0 + cn) * 8]
                    nc.gpsimd.dma_gather(
                        out_ap=m1[:, off + c0 : off + c0 + cn, :],
                        in_ap=taba, idxs_ap=idxs,
                        num_idxs=cn * P, num_idxs_reg=cn * P,
                        elem_size=A32, elem_step=A32,
                    )
                    nc.gpsimd.dma_gather(
                        out_ap=m0[:, off + c0 : off + c0 + cn, :].bitcast(f32),
                        in_ap=tab, idxs_ap=idxs,
                        num_idxs=cn * P, num_idxs_reg=cn * P,
                        elem_size=W32, elem_step=W32,
                    )
            m1b = m1[:].bitcast(bf16)
            mSb = mS[:].bitcast(bf16)
            # e = exp(lrelu(a_src + a_dst)); pairs-duplicated bf16
            s_t = scp.tile([P, cpbmax, K], f32)
            nc.vector.tensor_tensor(
                out=s_t[:, 0:nch, :],
                in0=m1b[:, 0:nch, 0:K],
                in1=mSb[:, 0:nch, BP:AUXW],
                op=mybir.AluOpType.add)
            lk = scp.tile([P, cpbmax, K], f32)
            nc.vector.scalar_tensor_tensor(
                out=lk[:, 0:nch, :], in0=s_t[:, 0:nch, :], scalar=NEG_SLOPE,
                in1=s_t[:, 0:nch, :],
                op0=mybir.AluOpType.mult, op1=mybir.AluOpType.max,
            )
            e2 = scp.tile([P, cpbmax, 2 * K], bf16)
            e2v = e2[:]
            for half in range(2):
                nc.scalar.activation(
                    out=bass.AP(tensor=e2v.tensor, offset=e2v.offset + half,
                                ap=[e2v.ap[0], [2 * K, nch], [2, K]]),
                    in_=lk[:, 0:nch, :],
                    func=mybir.ActivationFunctionType.Exp)
            # msgs *= bcast(e); (chunk,k)-flattened, last dims [1,2] -> 2x
            m0v = m0[:]
            nc.vector.tensor_tensor(
                out=bass.AP(tensor=m0v.tensor, offset=m0v.offset,
                            ap=[m0v.ap[0], [DK, nch * K],
                                [2, DK // 2], [1, 2]]),
                in0=bass.AP(tensor=m0v.tensor, offset=m0v.offset,
                            ap=[m0v.ap[0], [DK, nch * K],
                                [2, DK // 2], [1, 2]]),
                in1=bass.AP(tensor=e2v.tensor, offset=e2v.offset,
                            ap=[e2v.ap[0], [2, nch * K],
                                [0, DK // 2], [1, 2]]),
                op=mybir.AluOpType.mult,
            )
            # accumulate: acc[0:120] += sel^T msgs ; accE[0:120] += sel^T e2
            acc = accp.tile([P, OUT_DIM], f32)
            accE = accEp.tile([P, 2 * K], f32)
            for ci in range(nch):
                selci = mSb[:, ci, 0:BP]
                nc.tensor.matmul(
                    out=acc[0:BP, :],
                    lhsT=selci,
                    rhs=m0[:, ci, :],
                    start=(ci == 0),
                    stop=(ci == nch - 1),
                )
                nc.tensor.matmul(
                    out=accE[0:BP, :],
                    lhsT=selci,
                    rhs=e2[:, ci, :],
                    start=(ci == 0),
                    stop=(ci == nch - 1),
                )
            accEv = accE[:]
            r = smallp.tile([P, K], f32)
            nc.vector.tensor_scalar(
                out=r[0:BP, :],
                in0=bass.AP(tensor=accEv.tensor, offset=accEv.offset,
                            ap=[[accEv.ap[0][0], BP], [2, K]]),
                scalar1=1e-38, scalar2=None,
                op0=mybir.AluOpType.add,
            )
            nc.vector.reciprocal(out=r[0:BP, :], in_=r[0:BP, :])
            ot = outp.tile([P, OUT_DIM], f32)
            nc.vector.tensor_tensor(
                out=ot[0:BP, :], in0=acc[0:BP, :],
                in1=r[0:BP, :].to_broadcast([BP, K, DK]),
                op=mybir.AluOpType.mult,
            )
            nc.sync.dma_start(out=out_d.ap()[j * BP : (j + 1) * BP, :],
                              in_=ot[0:BP, :])
            cbA += na
            cbB += nb
            cbN += nch

    nc.compile()
    return nc


def run(h, edge_src, edge_dst, W, attn, n_cores=N_CORES, trace=False):
    import ml_dtypes
    from concourse.bass_utils import run_bass_kernel_spmd

    bf = ml_dtypes.bfloat16
    n_nodes = h.shape[0]
    h = np.asarray(h, dtype=np.float32)
    W = np.asarray(W, dtype=np.float32)
    attn = np.asarray(attn, dtype=np.float32)
    edge_src = np.asarray(edge_src)
    edge_dst = np.asarray(edge_dst)

    plan = build_plan(edge_src, edge_dst, n_nodes, n_cores)
    n_pad = plan["n_pad"]
    J = plan["J"]
    hTd = np.zeros((IN_DIM, n_pad), dtype=bf)
    hTd[:, :n_nodes] = h.T.astype(bf)
    c = (attn[:, :, None] * W.reshape(K, DK, IN_DIM)).sum(axis=1)
    waugT = np.concatenate([W.T, c.T], axis=1).astype(bf)
    # static one-hot rows for sdT: row j*BP+d = [onehot_120(d) | zeros(8)]
    soh1 = np.zeros((BP, 128), dtype=np.float32)
    soh1[:, :BP] = np.eye(BP, dtype=np.float32)
    soh = np.tile(soh1.astype(bf), (J, 1))

    nc = build_program(plan, n_cores)

    in_maps = []
    for cix in range(n_cores):
        in_maps.append({
            "hT": hTd,
            "waugT": waugT,
            "gA": plan["gA"][cix],
            "gB": plan["gB"][cix],
            "gS": plan["gS"][cix],
            "gRA": plan["gRA"][cix],
            "gRB": plan["gRB"][cix],
            "soh": soh,
        })
    try:
        res = run_bass_kernel_spmd(nc, in_maps, list(range(n_cores)), trace=trace)
    except Exception:
        if not trace:
            raise
        res = run_bass_kernel_spmd(nc, in_maps, list(range(n_cores)), trace=False)

    out_full = np.zeros((plan["B"] * BP, OUT_DIM), dtype=np.float32)
    for cix in range(n_cores):
        o = res.results[cix]["out"]
        for j in range(plan["J"]):
            b = plan["assign"][cix, j]
            if b >= 0:
                out_full[b * BP : (b + 1) * BP] = o[j * BP : (j + 1) * BP]
    out = out_full[:n_nodes].reshape(n_nodes, K, DK)
    return out, res


def kernel(h, edge_src, edge_dst, W, attn):
    out, _ = run(h, edge_src, edge_dst, W, attn)
    return out


# revision 9
# speedup vs baseline: 3.4486x; 1.0983x over previous
"""NeighborRoutingConv (GAT-style multi-head edge-softmax message passing) on 8 trn2 cores.

Strategy (v8, 120-dst blocks + merged sel/a_dst rows, f32-view gathers):
  - Host folds attn into the weight matrix (c = attn-weighted row sums of W).
  - Phase 1 (replicated): for all nodes compute Wh (256 bf16) -> whW table
    (512 B rows) and a (8 bf16) -> aux table (256 B rows), both with 64-row
    zero prologues per half so idx 0 is a safe zero row.
  - Blocks hold 120 destinations so a single 256 B row packs
    [one-hot(120) | a_dst(8)] bf16: one gather delivers both the scatter
    matrix column and the dst attention logits.
  - sdT table [ZP + J*120, 128 bf16]: static one-hot part copied DRAM->DRAM
    from a host constant; a_dst part (cols 120:128) written by a reorder pass
    that gathers each core's blocks' dst a-vectors from aux (A/B halves with
    zero-junk rows, combined by add).
  - Main loop (dst-sharded blocks, SPMD bin-packed; all gathers use f32
    bitcast views to halve the modeled element count):
      * m0 <- whW[src] rows (128 f32 elems)   = messages
      * m1 <- aux[src] rows (64 elems)        = a_src
      * mS <- sdT[j*120+dcol] rows (64 elems) = [sel one-hot | a_dst]
      * e = exp(lrelu(a_src + a_dst)) -> pair-duplicated bf16 (Act)
      * msgs *= bcast(e) via (chunk,k)-flattened [.,2] APs (DVE 2x mode)
      * per chunk: acc[0:120,0:256] += sel^T msgs ; accE[0:120,0:16] +=
        sel^T e_pairs (junk lanes hit zero rows and vanish)
      * out = acc * bcast(1/e_sum) -> DMA out f32.
"""

from contextlib import ExitStack

import numpy as np

P = 128
BP = 120  # dst nodes per block (one-hot width; +8 a_dst cols = 128)
IN_DIM = 256
OUT_DIM = 256
K = 8
DK = 32
RHS = OUT_DIM + K  # 264 phase-1 matmul width (Wh | a)
NEG_SLOPE = 0.2
N_CORES = 8
SUPER = 4  # node tiles per phase-1 iteration (512 nodes)
GMAX = 8  # chunks per dma_gather call (<=1024 descriptors)
ZP = 64  # zero-prologue rows per half


def _ceil_div(a, b):
    return (a + b - 1) // b


def _wrap16(lst):
    """dma_gather idx layout: [128, len//16] int16; idx i at [i%16, i//16],
    replicated across the 8 groups of 16 partitions."""
    n = len(lst)
    assert n % 16 == 0
    base = np.asarray(lst, dtype=np.int16).reshape(n // 16, 16).T  # [16, cols]
    return np.tile(base, (8, 1))  # [128, cols]


def build_plan(edge_src, edge_dst, n_nodes, n_cores):
    n_pad = _ceil_div(n_nodes, P * SUPER) * P * SUPER
    HALF = n_pad // 2
    B = _ceil_div(n_nodes, BP)
    J = _ceil_div(B, n_cores)

    perm = np.argsort(edge_dst, kind="stable")
    dsts = edge_dst[perm].astype(np.int64)
    srcs = edge_src[perm].astype(np.int64)
    bounds = np.searchsorted(dsts, np.arange(B + 1) * BP)

    # per-block A/B edge lists (split by src half for int16 gather indices)
    blkA, blkB = [], []
    for b in range(B):
        lo, hi = int(bounds[b]), int(bounds[b + 1])
        s, d = srcs[lo:hi], dsts[lo:hi]
        am = s < HALF
        blkA.append((s[am], d[am]))
        blkB.append((s[~am], d[~am]))

    chunksA = np.array([_ceil_div(max(len(blkA[b][0]), 1), P) for b in range(B)])
    chunksB = np.array([_ceil_div(max(len(blkB[b][0]), 1), P) for b in range(B)])

    # bin-pack: lexsort by (A, B) chunk counts desc, deal groups of n_cores
    order = np.lexsort((-chunksB, -chunksA))
    CPBA, CPBB = [], []
    assign = -np.ones((n_cores, J), dtype=np.int64)
    for j in range(J):
        grp = order[j * n_cores : (j + 1) * n_cores]
        CPBA.append(int(chunksA[grp].max()))
        CPBB.append(int(chunksB[grp].max()))
        for c, b in enumerate(grp):
            assign[c, j] = b
    NCH = [a + b for a, b in zip(CPBA, CPBB)]
    TOTCH = int(sum(NCH))
    TA = int(sum(CPBA))
    TB = int(sum(CPBB))
    JPAD = _ceil_div(J, GMAX) * GMAX

    gA = np.zeros((n_cores, P, TA * 8), dtype=np.int16)
    gB = np.zeros((n_cores, P, TB * 8), dtype=np.int16)
    gS = np.zeros((n_cores, P, TOTCH * 8), dtype=np.int16)  # sdT idx
    gRA = np.zeros((n_cores, P, JPAD * 8), dtype=np.int16)  # reorder A
    gRB = np.zeros((n_cores, P, JPAD * 8), dtype=np.int16)  # reorder B

    for c in range(n_cores):
        cbA = cbB = cbN = 0
        for j in range(J):
            na, nb, nch = CPBA[j], CPBB[j], NCH[j]
            b = assign[c, j]
            listA = np.zeros(na * P, dtype=np.int64)
            listB = np.zeros(nb * P, dtype=np.int64)
            listS = np.zeros(nch * P, dtype=np.int64)
            if b >= 0:
                base = b * BP
                sA, dA = blkA[b]
                sB, dB = blkB[b]
                listA[: len(sA)] = ZP + sA
                listB[: len(sB)] = ZP + (sB - HALF)
                # reorder gather: block's own 120 dst rows from aux (per-idx
                # half split; pad lanes 120..127 stay 0 -> zero rows)
                hdr = np.arange(BP) + base
                ra = np.where(hdr < HALF, ZP + hdr, 0)
                rb = np.where(hdr >= HALF, ZP + hdr - HALF, 0)
                gRA[c, :, j * 8 : (j + 1) * 8] = _wrap16(
                    np.concatenate([ra, np.zeros(P - BP, dtype=np.int64)]))
                gRB[c, :, j * 8 : (j + 1) * 8] = _wrap16(
                    np.concatenate([rb, np.zeros(P - BP, dtype=np.int64)]))
                # per-edge sdT idx (segment A then B)
                for lst_d, off, ne in ((dA, 0, len(sA)), (dB, na * P, len(sB))):
                    if ne:
                        s_ = np.arange(ne)
                        listS[off + s_] = ZP + j * BP + (lst_d - base)
            gA[c, :, cbA * 8 : (cbA + na) * 8] = _wrap16(listA)
            gB[c, :, cbB * 8 : (cbB + nb) * 8] = _wrap16(listB)
            gS[c, :, cbN * 8 : (cbN + nch) * 8] = _wrap16(listS)
            cbA += na
            cbB += nb
            cbN += nch

    return {
        "n_pad": n_pad,
        "HALF": HALF,
        "B": B,
        "J": J,
        "JPAD": JPAD,
        "CPBA": CPBA,
        "CPBB": CPBB,
        "NCH": NCH,
        "TOTCH": TOTCH,
        "TA": TA,
        "TB": TB,
        "CPBMAX": max(NCH),
        "assign": assign,
        "gA": gA,
        "gB": gB,
        "gS": gS,
        "gRA": gRA,
        "gRB": gRB,
    }


def build_program(plan, n_cores):
    import concourse.bass as bass
    import concourse.tile as tile
    from concourse import bacc, mybir

    f32 = mybir.dt.float32
    bf16 = mybir.dt.bfloat16
    i16 = mybir.dt.int16

    n_pad = plan["n_pad"]
    HALF = plan["HALF"]
    J = plan["J"]
    JPAD = plan["JPAD"]
    CPBA, CPBB, NCH = plan["CPBA"], plan["CPBB"], plan["NCH"]
    TOTCH, TA, TB = plan["TOTCH"], plan["TA"], plan["TB"]
    cpbmax = plan["CPBMAX"]
    NT = n_pad // (P * SUPER)
    NT_HALF = HALF // (P * SUPER)
    CG = IN_DIM // P
    AUXW = 128  # aux/sdT row width (bf16): 256 B rows
    A32 = AUXW // 2  # 64 f32 per aux/sdT row
    W32 = OUT_DIM // 2  # 128 f32 per whW row

    nc = bacc.Bacc("TRN2", target_bir_lowering=False, debug=False,
                   num_devices=n_cores)

    hT = nc.dram_tensor("hT", [IN_DIM, n_pad], bf16, kind="ExternalInput")
    waugT = nc.dram_tensor("waugT", [IN_DIM, RHS], bf16, kind="ExternalInput")
    gA_d = nc.dram_tensor("gA", [P, TA * 8], i16, kind="ExternalInput")
    gB_d = nc.dram_tensor("gB", [P, TB * 8], i16, kind="ExternalInput")
    gS_d = nc.dram_tensor("gS", [P, TOTCH * 8], i16, kind="ExternalInput")
    gRA_d = nc.dram_tensor("gRA", [P, JPAD * 8], i16, kind="ExternalInput")
    gRB_d = nc.dram_tensor("gRB", [P, JPAD * 8], i16, kind="ExternalInput")
    soh_d = nc.dram_tensor("soh", [J * BP, AUXW], bf16, kind="ExternalInput")
    out_d = nc.dram_tensor("out", [J * BP, OUT_DIM], f32, kind="ExternalOutput")
    # device tables: [zero x64 | A-half | zero x64 | B-half]
    whW = nc.dram_tensor("whW", [n_pad + 2 * ZP, OUT_DIM], bf16)
    aux = nc.dram_tensor("aux", [n_pad + 2 * ZP, AUXW], bf16)
    sdT = nc.dram_tensor("sdT", [ZP + J * BP, AUXW], bf16)

    with tile.TileContext(nc) as tc, ExitStack() as ctx:
        consts = ctx.enter_context(tc.tile_pool(name="consts", bufs=1))
        ctx1 = ctx.enter_context(ExitStack())
        p1in = ctx1.enter_context(tc.tile_pool(name="p1in", bufs=6))
        p1ps = ctx1.enter_context(tc.tile_pool(name="p1ps", bufs=4, space="PSUM"))
        p1w = ctx1.enter_context(tc.tile_pool(name="p1w", bufs=8))
        p1w2 = ctx1.enter_context(tc.tile_pool(name="p1w2", bufs=8))
        p1a = ctx1.enter_context(tc.tile_pool(name="p1a", bufs=6))

        waug_sb = consts.tile([P, CG, RHS], bf16)
        nc.sync.dma_start(out=waug_sb[:],
                          in_=waugT.ap().rearrange("(g p) r -> p g r", p=P))
        gA_sb = consts.tile([P, TA * 8], i16)
        nc.scalar.dma_start(out=gA_sb[:], in_=gA_d.ap())
        gB_sb = consts.tile([P, TB * 8], i16)
        nc.scalar.dma_start(out=gB_sb[:], in_=gB_d.ap())
        gS_sb = consts.tile([P, TOTCH * 8], i16)
        nc.scalar.dma_start(out=gS_sb[:], in_=gS_d.ap())
        gRA_sb = consts.tile([P, JPAD * 8], i16)
        nc.scalar.dma_start(out=gRA_sb[:], in_=gRA_d.ap())
        gRB_sb = consts.tile([P, JPAD * 8], i16)
        nc.scalar.dma_start(out=gRB_sb[:], in_=gRB_d.ap())

        # zero prologues + static one-hot part of sdT (DRAM->DRAM)
        zt = consts.tile([P, OUT_DIM], bf16)
        nc.vector.memset(zt[:], 0.0)
        for half in range(2):
            r0 = half * (HALF + ZP)
            nc.scalar.dma_start(out=whW.ap()[r0 : r0 + ZP, :], in_=zt[0:ZP, :])
            nc.scalar.dma_start(out=aux.ap()[r0 : r0 + ZP, :],
                                in_=zt[0:ZP, 0:AUXW])
        nc.scalar.dma_start(out=sdT.ap()[0:ZP, :], in_=zt[0:ZP, 0:AUXW])
        nc.scalar.dma_start(
            out=sdT.ap()[ZP : ZP + J * BP, :].rearrange(
                "(j b) w -> b j w", b=BP),
            in_=soh_d.ap().rearrange("(j b) w -> b j w", b=BP))

        # ---- phase 1: whW + aux tables for all nodes ----
        hT_r = hT.ap().rearrange("(g p) n -> p g n", p=P)
        # group phase-1 iters (<=4, never straddling the half boundary) so the
        # tiny aux writes amortize the per-DMA floor
        groups = []
        it = 0
        while it < NT:
            n = min(4, NT - it, (NT_HALF - it) if it < NT_HALF else NT)
            groups.append((it, n))
            it += n
        for g0, gn in groups:
            stA4 = p1a.tile([P, 4 * SUPER, K], bf16)
            for gi in range(gn):
                it = g0 + gi
                r0 = it * SUPER * P + (ZP if it < NT_HALF else 2 * ZP)
                ht = p1in.tile([P, CG, SUPER * P], bf16)
                nc.sync.dma_start(
                    out=ht[:],
                    in_=hT_r[:, :, it * SUPER * P : (it + 1) * SUPER * P]
                )
                whr = whW.ap()[r0 : r0 + SUPER * P, :].rearrange(
                    "(t p) r -> p t r", p=P)
                for h in range(2):
                    # half-size PSUM tiles (4-deep rotation) with alternating
                    # copier engines: finer release granularity for the PE
                    ps = p1ps.tile([P, 2, 512], f32)
                    for t in range(2):
                        for g in range(CG):
                            nc.tensor.matmul(
                                out=ps[:, t, 0:RHS],
                                lhsT=ht[:, g, (h * 2 + t) * P : (h * 2 + t + 1) * P],
                                rhs=waug_sb[:, g, :],
                                start=(g == 0),
                                stop=(g == CG - 1),
                            )
                    st = p1w.tile([P, 2, OUT_DIM], bf16)
                    sta = stA4[:, gi * SUPER + h * 2 : gi * SUPER + h * 2 + 2, :]
                    if h == 0:
                        nc.scalar.copy(out=st[:], in_=ps[:, :, 0:OUT_DIM])
                    else:
                        nc.vector.tensor_copy(out=st[:], in_=ps[:, :, 0:OUT_DIM])
                    nc.vector.tensor_copy(out=sta, in_=ps[:, :, OUT_DIM:RHS])
                    nc.gpsimd.dma_start(out=whr[:, h * 2 : h * 2 + 2, :],
                                        in_=st[:])
            ra = g0 * SUPER * P + (ZP if g0 < NT_HALF else 2 * ZP)
            nc.sync.dma_start(
                out=aux.ap()[ra : ra + gn * SUPER * P, 0:K].rearrange(
                    "(t p) k -> p t k", p=P),
                in_=stA4[:, 0 : gn * SUPER, :])

        ctx1.close()
        tc.strict_bb_all_engine_barrier()

        # ---- reorder: sdT[ZP + j*120 + d, 120:128] = a of node b*120+d ----
        ctx2 = ctx.enter_context(ExitStack())
        rp = ctx2.enter_context(tc.tile_pool(name="rp", bufs=1))
        auxA = aux.ap()[0 : ZP + HALF, :].bitcast(f32)
        auxB = aux.ap()[ZP + HALF : n_pad + 2 * ZP, :].bitcast(f32)
        rA = rp.tile([P, JPAD, A32], f32)
        rB = rp.tile([P, JPAD, A32], f32)
        for c0 in range(0, JPAD, GMAX):
            cn = min(GMAX, JPAD - c0)
            for tab, gsb, dst in ((auxA, gRA_sb, rA), (auxB, gRB_sb, rB)):
                nc.gpsimd.dma_gather(
                    out_ap=dst[:, c0 : c0 + cn, :],
                    in_ap=tab,
                    idxs_ap=gsb[:, c0 * 8 : (c0 + cn) * 8],
                    num_idxs=cn * P,
                    num_idxs_reg=cn * P,
                    elem_size=A32,
                    elem_step=A32,
                )
        rab = rp.tile([P, J, K], bf16)
        nc.vector.tensor_tensor(
            out=rab[:],
            in0=rA[:].bitcast(bf16)[:, 0:J, 0:K],
            in1=rB[:].bitcast(bf16)[:, 0:J, 0:K],
            op=mybir.AluOpType.add,
        )
        nc.sync.dma_start(
            out=sdT.ap()[ZP : ZP + J * BP, BP:AUXW].rearrange(
                "(j p) k -> p j k", p=BP),
            in_=rab[0:BP, :, :])
        ctx2.close()
        tc.strict_bb_all_engine_barrier()

        # ---- main loop ----
        m0p = ctx.enter_context(tc.tile_pool(name="m0p", bufs=4))
        m1p = ctx.enter_context(tc.tile_pool(name="m1p", bufs=4))
        mSp = ctx.enter_context(tc.tile_pool(name="mSp", bufs=4))
        scp = ctx.enter_context(tc.tile_pool(name="scp", bufs=4))
        accp = ctx.enter_context(tc.tile_pool(name="accp", bufs=3, space="PSUM"))
        accEp = ctx.enter_context(tc.tile_pool(name="accEp", bufs=3, space="PSUM"))
        outp = ctx.enter_context(tc.tile_pool(name="outp", bufs=3))
        smallp = ctx.enter_context(tc.tile_pool(name="smallp", bufs=4))

        whA = whW.ap()[0 : ZP + HALF, :].bitcast(f32)
        whB = whW.ap()[ZP + HALF : n_pad + 2 * ZP, :].bitcast(f32)
        sdTf = sdT.ap().bitcast(f32)

        cbA = cbB = cbN = 0
        for j in range(J):
            na, nb, nch = CPBA[j], CPBB[j], NCH[j]
            m0 = m0p.tile([P, cpbmax, OUT_DIM], bf16)
            m1 = m1p.tile([P, cpbmax, A32], f32)
            mS = mSp.tile([P, cpbmax, A32], f32)
            # dst-side gather first: [one-hot | a_dst] rows from sdT
            for c0 in range(0, nch, GMAX):
                cn = min(GMAX, nch - c0)
                nc.gpsimd.dma_gather(
                    out_ap=mS[:, c0 : c0 + cn, :],
                    in_ap=sdTf,
                    idxs_ap=gS_sb[:, (cbN + c0) * 8 : (cbN + c0 + cn) * 8],
                    num_idxs=cn * P, num_idxs_reg=cn * P,
                    elem_size=A32, elem_step=A32,
                )
            # src gathers: a_src rows then Wh rows, A then B segments
            for tab, taba, nseg, cb, gsb, off in (
                (whA, auxA, na, cbA, gA_sb, 0),
                (whB, auxB, nb, cbB, gB_sb, na),
            ):
                for c0 in range(0, nseg, GMAX):
                    cn = min(GMAX, nseg - c0)
                    idxs = gsb[:, (cb + c0) * 8 : (cb + c0 + cn) * 8]
                    nc.gpsimd.dma_gather(
                        out_ap=m1[:, off + c0 : off + c0 + cn, :],
                        in_ap=taba, idxs_ap=idxs,
                        num_idxs=cn * P, num_idxs_reg=cn * P,
                        elem_size=A32, elem_step=A32,
                    )
                    nc.gpsimd.dma_gather(
                        out_ap=m0[:, off + c0 : off + c0 + cn, :].bitcast(f32),
                        in_ap=tab, idxs_ap=idxs,
                        num_idxs=cn * P, num_idxs_reg=cn * P,
                        elem_size=W32, elem_step=W32,
                    )
            m1b = m1[:].bitcast(bf16)
            mSb = mS[:].bitcast(bf16)
            # e = exp(lrelu(a_src + a_dst)); pairs-duplicated bf16
            s_t = scp.tile([P, cpbmax, K], f32)
            nc.vector.tensor_tensor(
                out=s_t[:, 0:nch, :],
                in0=m1b[:, 0:nch, 0:K],
                in1=mSb[:, 0:nch, BP:AUXW],
                op=mybir.AluOpType.add)
            lk = scp.tile([P, cpbmax, K], f32)
            nc.vector.scalar_tensor_tensor(
                out=lk[:, 0:nch, :], in0=s_t[:, 0:nch, :], scalar=NEG_SLOPE,
                in1=s_t[:, 0:nch, :],
                op0=mybir.AluOpType.mult, op1=mybir.AluOpType.max,
            )
            e2 = scp.tile([P, cpbmax, 2 * K], bf16)
            e2v = e2[:]
            for half in range(2):
                nc.scalar.activation(
                    out=bass.AP(tensor=e2v.tensor, offset=e2v.offset + half,
                                ap=[e2v.ap[0], [2 * K, nch], [2, K]]),
                    in_=lk[:, 0:nch, :],
                    func=mybir.ActivationFunctionType.Exp)
            # msgs *= bcast(e); (chunk,k)-flattened, last dims [1,2] -> 2x
            m0v = m0[:]
            nc.vector.tensor_tensor(
                out=bass.AP(tensor=m0v.tensor, offset=m0v.offset,
                            ap=[m0v.ap[0], [DK, nch * K],
                                [2, DK // 2], [1, 2]]),
                in0=bass.AP(tensor=m0v.tensor, offset=m0v.offset,
                            ap=[m0v.ap[0], [DK, nch * K],
                                [2, DK // 2], [1, 2]]),
                in1=bass.AP(tensor=e2v.tensor, offset=e2v.offset,
                            ap=[e2v.ap[0], [2, nch * K],
                                [0, DK // 2], [1, 2]]),
                op=mybir.AluOpType.mult,
            )
            # accumulate: acc[0:120] += sel^T msgs ; accE[0:120] += sel^T e2
            acc = accp.tile([P, OUT_DIM], f32)
            accE = accEp.tile([P, 2 * K], f32)
            for ci in range(nch):
                selci = mSb[:, ci, 0:BP]
                nc.tensor.matmul(
                    out=acc[0:BP, :],
                    lhsT=selci,
                    rhs=m0[:, ci, :],
                    start=(ci == 0),
                    stop=(ci == nch - 1),
                )
                nc.tensor.matmul(
                    out=accE[0:BP, :],
                    lhsT=selci,
                    rhs=e2[:, ci, :],
                    start=(ci == 0),
                    stop=(ci == nch - 1),
                )
            accEv = accE[:]
            r = smallp.tile([P, K], f32)
            nc.scalar.activation(
                out=r[0:BP, :],
                in_=bass.AP(tensor=accEv.tensor, offset=accEv.offset,
                            ap=[[accEv.ap[0][0], BP], [2, K]]),
                func=mybir.ActivationFunctionType.Copy, bias=1e-38,
            )
            nc.vector.reciprocal(out=r[0:BP, :], in_=r[0:BP, :])
            ot = outp.tile([P, OUT_DIM], f32)
            nc.vector.tensor_tensor(
                out=ot[0:BP, :], in0=acc[0:BP, :],
                in1=r[0:BP, :].to_broadcast([BP, K, DK]),
                op=mybir.AluOpType.mult,
            )
            nc.sync.dma_start(out=out_d.ap()[j * BP : (j + 1) * BP, :],
                              in_=ot[0:BP, :])
            cbA += na
            cbB += nb
            cbN += nch

    nc.compile()
    return nc


def run(h, edge_src, edge_dst, W, attn, n_cores=N_CORES, trace=False):
    import ml_dtypes
    from concourse.bass_utils import run_bass_kernel_spmd

    bf = ml_dtypes.bfloat16
    n_nodes = h.shape[0]
    h = np.asarray(h, dtype=np.float32)
    W = np.asarray(W, dtype=np.float32)
    attn = np.asarray(attn, dtype=np.float32)
    edge_src = np.asarray(edge_src)
    edge_dst = np.asarray(edge_dst)

    plan = build_plan(edge_src, edge_dst, n_nodes, n_cores)
    n_pad = plan["n_pad"]
    J = plan["J"]
    hTd = np.zeros((IN_DIM, n_pad), dtype=bf)
    hTd[:, :n_nodes] = h.T.astype(bf)
    c = (attn[:, :, None] * W.reshape(K, DK, IN_DIM)).sum(axis=1)
    waugT = np.concatenate([W.T, c.T], axis=1).astype(bf)
    # static one-hot rows for sdT: row j*BP+d = [onehot_120(d) | zeros(8)]
    soh1 = np.zeros((BP, 128), dtype=np.float32)
    soh1[:, :BP] = np.eye(BP, dtype=np.float32)
    soh = np.tile(soh1.astype(bf), (J, 1))

    nc = build_program(plan, n_cores)

    in_maps = []
    for cix in range(n_cores):
        in_maps.append({
            "hT": hTd,
            "waugT": waugT,
            "gA": plan["gA"][cix],
            "gB": plan["gB"][cix],
            "gS": plan["gS"][cix],
            "gRA": plan["gRA"][cix],
            "gRB": plan["gRB"][cix],
            "soh": soh,
        })
    try:
        res = run_bass_kernel_spmd(nc, in_maps, list(range(n_cores)), trace=trace)
    except Exception:
        if not trace:
            raise
        res = run_bass_kernel_spmd(nc, in_maps, list(range(n_cores)), trace=False)

    out_full = np.zeros((plan["B"] * BP, OUT_DIM), dtype=np.float32)
    for cix in range(n_cores):
        o = res.results[cix]["out"]
        for j in range(plan["J"]):
            b = plan["assign"][cix, j]
            if b >= 0:
                out_full[b * BP : (b + 1) * BP] = o[j * BP : (j + 1) * BP]
    out = out_full[:n_nodes].reshape(n_nodes, K, DK)
    return out, res


def kernel(h, edge_src, edge_dst, W, attn):
    out, _ = run(h, edge_src, edge_dst, W, attn)
    return out


# revision 10
# speedup vs baseline: 3.5039x; 1.0160x over previous
"""NeighborRoutingConv (GAT-style multi-head edge-softmax message passing) on 8 trn2 cores.

Strategy (v8, 120-dst blocks + merged sel/a_dst rows, f32-view gathers):
  - Host folds attn into the weight matrix (c = attn-weighted row sums of W).
  - Phase 1 (replicated): for all nodes compute Wh (256 bf16) -> whW table
    (512 B rows) and a (8 bf16) -> aux table (256 B rows), both with 64-row
    zero prologues per half so idx 0 is a safe zero row.
  - Blocks hold 120 destinations so a single 256 B row packs
    [one-hot(120) | a_dst(8)] bf16: one gather delivers both the scatter
    matrix column and the dst attention logits.
  - sdT table [ZP + J*120, 128 bf16]: static one-hot part copied DRAM->DRAM
    from a host constant; a_dst part (cols 120:128) written by a reorder pass
    that gathers each core's blocks' dst a-vectors from aux (A/B halves with
    zero-junk rows, combined by add).
  - Main loop (dst-sharded blocks, SPMD bin-packed; all gathers use f32
    bitcast views to halve the modeled element count):
      * m0 <- whW[src] rows (128 f32 elems)   = messages
      * m1 <- aux[src] rows (64 elems)        = a_src
      * mS <- sdT[j*120+dcol] rows (64 elems) = [sel one-hot | a_dst]
      * e = exp(lrelu(a_src + a_dst)) -> pair-duplicated bf16 (Act)
      * msgs *= bcast(e) via (chunk,k)-flattened [.,2] APs (DVE 2x mode)
      * per chunk: acc[0:120,0:256] += sel^T msgs ; accE[0:120,0:16] +=
        sel^T e_pairs (junk lanes hit zero rows and vanish)
      * out = acc * bcast(1/e_sum) -> DMA out f32.
"""

from contextlib import ExitStack

import numpy as np

P = 128
BP = 120  # dst nodes per block (one-hot width; +8 a_dst cols = 128)
IN_DIM = 256
OUT_DIM = 256
K = 8
DK = 32
RHS = OUT_DIM + K  # 264 phase-1 matmul width (Wh | a)
NEG_SLOPE = 0.2
N_CORES = 8
SUPER = 4  # node tiles per phase-1 iteration (512 nodes)
GMAX = 8  # chunks per dma_gather call (<=1024 descriptors)
ZP = 64  # zero-prologue rows per half


def _ceil_div(a, b):
    return (a + b - 1) // b


def _wrap16(lst):
    """dma_gather idx layout: [128, len//16] int16; idx i at [i%16, i//16],
    replicated across the 8 groups of 16 partitions."""
    n = len(lst)
    assert n % 16 == 0
    base = np.asarray(lst, dtype=np.int16).reshape(n // 16, 16).T  # [16, cols]
    return np.tile(base, (8, 1))  # [128, cols]


def build_plan(edge_src, edge_dst, n_nodes, n_cores):
    n_pad = _ceil_div(n_nodes, P * SUPER) * P * SUPER
    HALF = n_pad // 2
    B = _ceil_div(n_nodes, BP)
    J = _ceil_div(B, n_cores)

    perm = np.argsort(edge_dst, kind="stable")
    dsts = edge_dst[perm].astype(np.int64)
    srcs = edge_src[perm].astype(np.int64)
    bounds = np.searchsorted(dsts, np.arange(B + 1) * BP)

    # per-block A/B edge lists (split by src half for int16 gather indices)
    blkA, blkB = [], []
    for b in range(B):
        lo, hi = int(bounds[b]), int(bounds[b + 1])
        s, d = srcs[lo:hi], dsts[lo:hi]
        am = s < HALF
        blkA.append((s[am], d[am]))
        blkB.append((s[~am], d[~am]))

    chunksA = np.array([_ceil_div(max(len(blkA[b][0]), 1), P) for b in range(B)])
    chunksB = np.array([_ceil_div(max(len(blkB[b][0]), 1), P) for b in range(B)])

    # bin-pack: lexsort by (A, B) chunk counts desc, deal groups of n_cores
    order = np.lexsort((-chunksB, -chunksA))
    CPBA, CPBB = [], []
    assign = -np.ones((n_cores, J), dtype=np.int64)
    for j in range(J):
        grp = order[j * n_cores : (j + 1) * n_cores]
        CPBA.append(int(chunksA[grp].max()))
        CPBB.append(int(chunksB[grp].max()))
        for c, b in enumerate(grp):
            assign[c, j] = b
    NCH = [a + b for a, b in zip(CPBA, CPBB)]
    TOTCH = int(sum(NCH))
    TA = int(sum(CPBA))
    TB = int(sum(CPBB))
    JPAD = _ceil_div(J, GMAX) * GMAX

    gA = np.zeros((n_cores, P, TA * 8), dtype=np.int16)
    gB = np.zeros((n_cores, P, TB * 8), dtype=np.int16)
    gS = np.zeros((n_cores, P, TOTCH * 8), dtype=np.int16)  # sdT idx
    gRA = np.zeros((n_cores, P, JPAD * 8), dtype=np.int16)  # reorder A
    gRB = np.zeros((n_cores, P, JPAD * 8), dtype=np.int16)  # reorder B

    for c in range(n_cores):
        cbA = cbB = cbN = 0
        for j in range(J):
            na, nb, nch = CPBA[j], CPBB[j], NCH[j]
            b = assign[c, j]
            listA = np.zeros(na * P, dtype=np.int64)
            listB = np.zeros(nb * P, dtype=np.int64)
            listS = np.zeros(nch * P, dtype=np.int64)
            if b >= 0:
                base = b * BP
                sA, dA = blkA[b]
                sB, dB = blkB[b]
                listA[: len(sA)] = ZP + sA
                listB[: len(sB)] = ZP + (sB - HALF)
                # reorder gather: block's own 120 dst rows from aux (per-idx
                # half split; pad lanes 120..127 stay 0 -> zero rows)
                hdr = np.arange(BP) + base
                ra = np.where(hdr < HALF, ZP + hdr, 0)
                rb = np.where(hdr >= HALF, ZP + hdr - HALF, 0)
                gRA[c, :, j * 8 : (j + 1) * 8] = _wrap16(
                    np.concatenate([ra, np.zeros(P - BP, dtype=np.int64)]))
                gRB[c, :, j * 8 : (j + 1) * 8] = _wrap16(
                    np.concatenate([rb, np.zeros(P - BP, dtype=np.int64)]))
                # per-edge sdT idx (segment A then B)
                for lst_d, off, ne in ((dA, 0, len(sA)), (dB, na * P, len(sB))):
                    if ne:
                        s_ = np.arange(ne)
                        listS[off + s_] = ZP + j * BP + (lst_d - base)
            gA[c, :, cbA * 8 : (cbA + na) * 8] = _wrap16(listA)
            gB[c, :, cbB * 8 : (cbB + nb) * 8] = _wrap16(listB)
            gS[c, :, cbN * 8 : (cbN + nch) * 8] = _wrap16(listS)
            cbA += na
            cbB += nb
            cbN += nch

    return {
        "n_pad": n_pad,
        "HALF": HALF,
        "B": B,
        "J": J,
        "JPAD": JPAD,
        "CPBA": CPBA,
        "CPBB": CPBB,
        "NCH": NCH,
        "TOTCH": TOTCH,
        "TA": TA,
        "TB": TB,
        "CPBMAX": max(NCH),
        "assign": assign,
        "gA": gA,
        "gB": gB,
        "gS": gS,
        "gRA": gRA,
        "gRB": gRB,
    }


def build_program(plan, n_cores):
    import concourse.bass as bass
    import concourse.tile as tile
    from concourse import bacc, mybir

    f32 = mybir.dt.float32
    bf16 = mybir.dt.bfloat16
    i16 = mybir.dt.int16

    n_pad = plan["n_pad"]
    HALF = plan["HALF"]
    J = plan["J"]
    JPAD = plan["JPAD"]
    CPBA, CPBB, NCH = plan["CPBA"], plan["CPBB"], plan["NCH"]
    TOTCH, TA, TB = plan["TOTCH"], plan["TA"], plan["TB"]
    cpbmax = plan["CPBMAX"]
    NT = n_pad // (P * SUPER)
    NT_HALF = HALF // (P * SUPER)
    CG = IN_DIM // P
    AUXW = 128  # aux/sdT row width (bf16): 256 B rows
    A32 = AUXW // 2  # 64 f32 per aux/sdT row
    W32 = OUT_DIM // 2  # 128 f32 per whW row

    nc = bacc.Bacc("TRN2", target_bir_lowering=False, debug=False,
                   num_devices=n_cores)

    hT = nc.dram_tensor("hT", [IN_DIM, n_pad], bf16, kind="ExternalInput")
    waugT = nc.dram_tensor("waugT", [IN_DIM, RHS], bf16, kind="ExternalInput")
    gA_d = nc.dram_tensor("gA", [P, TA * 8], i16, kind="ExternalInput")
    gB_d = nc.dram_tensor("gB", [P, TB * 8], i16, kind="ExternalInput")
    gS_d = nc.dram_tensor("gS", [P, TOTCH * 8], i16, kind="ExternalInput")
    gRA_d = nc.dram_tensor("gRA", [P, JPAD * 8], i16, kind="ExternalInput")
    gRB_d = nc.dram_tensor("gRB", [P, JPAD * 8], i16, kind="ExternalInput")
    soh_d = nc.dram_tensor("soh", [J * BP, AUXW], bf16, kind="ExternalInput")
    out_d = nc.dram_tensor("out", [J * BP, OUT_DIM], f32, kind="ExternalOutput")
    # device tables: [zero x64 | A-half | zero x64 | B-half]
    whW = nc.dram_tensor("whW", [n_pad + 2 * ZP, OUT_DIM], bf16)
    aux = nc.dram_tensor("aux", [n_pad + 2 * ZP, AUXW], bf16)
    sdT = nc.dram_tensor("sdT", [ZP + J * BP, AUXW], bf16)

    with tile.TileContext(nc) as tc, ExitStack() as ctx:
        consts = ctx.enter_context(tc.tile_pool(name="consts", bufs=1))
        ctx1 = ctx.enter_context(ExitStack())
        p1in = ctx1.enter_context(tc.tile_pool(name="p1in", bufs=6))
        p1ps = ctx1.enter_context(tc.tile_pool(name="p1ps", bufs=4, space="PSUM"))
        p1w = ctx1.enter_context(tc.tile_pool(name="p1w", bufs=8))
        p1w2 = ctx1.enter_context(tc.tile_pool(name="p1w2", bufs=8))
        p1a = ctx1.enter_context(tc.tile_pool(name="p1a", bufs=6))

        waug_sb = consts.tile([P, CG, RHS], bf16)
        nc.sync.dma_start(out=waug_sb[:],
                          in_=waugT.ap().rearrange("(g p) r -> p g r", p=P))
        gA_sb = consts.tile([P, TA * 8], i16)
        nc.scalar.dma_start(out=gA_sb[:], in_=gA_d.ap())
        gB_sb = consts.tile([P, TB * 8], i16)
        nc.scalar.dma_start(out=gB_sb[:], in_=gB_d.ap())
        gS_sb = consts.tile([P, TOTCH * 8], i16)
        nc.scalar.dma_start(out=gS_sb[:], in_=gS_d.ap())
        gRA_sb = consts.tile([P, JPAD * 8], i16)
        nc.scalar.dma_start(out=gRA_sb[:], in_=gRA_d.ap())
        gRB_sb = consts.tile([P, JPAD * 8], i16)
        nc.scalar.dma_start(out=gRB_sb[:], in_=gRB_d.ap())

        # zero prologues + static one-hot part of sdT (DRAM->DRAM)
        zt = consts.tile([P, OUT_DIM], bf16)
        nc.vector.memset(zt[:], 0.0)
        for half in range(2):
            r0 = half * (HALF + ZP)
            nc.scalar.dma_start(out=whW.ap()[r0 : r0 + ZP, :], in_=zt[0:ZP, :])
            nc.scalar.dma_start(out=aux.ap()[r0 : r0 + ZP, :],
                                in_=zt[0:ZP, 0:AUXW])
        nc.scalar.dma_start(out=sdT.ap()[0:ZP, :], in_=zt[0:ZP, 0:AUXW])
        nc.scalar.dma_start(
            out=sdT.ap()[ZP : ZP + J * BP, :].rearrange(
                "(j b) w -> b j w", b=BP),
            in_=soh_d.ap().rearrange("(j b) w -> b j w", b=BP))

        # ---- phase 1: whW + aux tables for all nodes ----
        hT_r = hT.ap().rearrange("(g p) n -> p g n", p=P)
        # group phase-1 iters (<=4, never straddling the half boundary) so the
        # tiny aux writes amortize the per-DMA floor
        groups = []
        it = 0
        while it < NT:
            n = min(4, NT - it, (NT_HALF - it) if it < NT_HALF else NT)
            groups.append((it, n))
            it += n
        for g0, gn in groups:
            stA4 = p1a.tile([P, 4 * SUPER, K], bf16)
            for gi in range(gn):
                it = g0 + gi
                r0 = it * SUPER * P + (ZP if it < NT_HALF else 2 * ZP)
                ht = p1in.tile([P, CG, SUPER * P], bf16)
                nc.sync.dma_start(
                    out=ht[:],
                    in_=hT_r[:, :, it * SUPER * P : (it + 1) * SUPER * P]
                )
                whr = whW.ap()[r0 : r0 + SUPER * P, :].rearrange(
                    "(t p) r -> p t r", p=P)
                st = p1w.tile([P, SUPER, OUT_DIM], bf16)
                for h in range(2):
                    # half-size PSUM tiles (4-deep rotation); one copier
                    # engine per ITERATION so the single whW write per iter
                    # avoids the 500 ns per-DMA floor
                    ps = p1ps.tile([P, 2, 512], f32)
                    for t in range(2):
                        for g in range(CG):
                            nc.tensor.matmul(
                                out=ps[:, t, 0:RHS],
                                lhsT=ht[:, g, (h * 2 + t) * P : (h * 2 + t + 1) * P],
                                rhs=waug_sb[:, g, :],
                                start=(g == 0),
                                stop=(g == CG - 1),
                            )
                    sta = stA4[:, gi * SUPER + h * 2 : gi * SUPER + h * 2 + 2, :]
                    if it % 2 == 0:
                        nc.scalar.copy(out=st[:, h * 2 : h * 2 + 2, :],
                                       in_=ps[:, :, 0:OUT_DIM])
                    else:
                        nc.vector.tensor_copy(out=st[:, h * 2 : h * 2 + 2, :],
                                              in_=ps[:, :, 0:OUT_DIM])
                    nc.vector.tensor_copy(out=sta, in_=ps[:, :, OUT_DIM:RHS])
                nc.gpsimd.dma_start(out=whr[:], in_=st[:])
            ra = g0 * SUPER * P + (ZP if g0 < NT_HALF else 2 * ZP)
            nc.sync.dma_start(
                out=aux.ap()[ra : ra + gn * SUPER * P, 0:K].rearrange(
                    "(t p) k -> p t k", p=P),
                in_=stA4[:, 0 : gn * SUPER, :])

        ctx1.close()
        tc.strict_bb_all_engine_barrier()

        # ---- reorder: sdT[ZP + j*120 + d, 120:128] = a of node b*120+d ----
        ctx2 = ctx.enter_context(ExitStack())
        rp = ctx2.enter_context(tc.tile_pool(name="rp", bufs=1))
        auxA = aux.ap()[0 : ZP + HALF, :].bitcast(f32)
        auxB = aux.ap()[ZP + HALF : n_pad + 2 * ZP, :].bitcast(f32)
        rA = rp.tile([P, JPAD, A32], f32)
        rB = rp.tile([P, JPAD, A32], f32)
        for c0 in range(0, JPAD, GMAX):
            cn = min(GMAX, JPAD - c0)
            for tab, gsb, dst in ((auxA, gRA_sb, rA), (auxB, gRB_sb, rB)):
                nc.gpsimd.dma_gather(
                    out_ap=dst[:, c0 : c0 + cn, :],
                    in_ap=tab,
                    idxs_ap=gsb[:, c0 * 8 : (c0 + cn) * 8],
                    num_idxs=cn * P,
                    num_idxs_reg=cn * P,
                    elem_size=A32,
                    elem_step=A32,
                )
        rab = rp.tile([P, J, K], bf16)
        nc.vector.tensor_tensor(
            out=rab[:],
            in0=rA[:].bitcast(bf16)[:, 0:J, 0:K],
            in1=rB[:].bitcast(bf16)[:, 0:J, 0:K],
            op=mybir.AluOpType.add,
        )
        nc.sync.dma_start(
            out=sdT.ap()[ZP : ZP + J * BP, BP:AUXW].rearrange(
                "(j p) k -> p j k", p=BP),
            in_=rab[0:BP, :, :])
        ctx2.close()
        tc.strict_bb_all_engine_barrier()

        # ---- main loop ----
        m0p = ctx.enter_context(tc.tile_pool(name="m0p", bufs=5))
        m1p = ctx.enter_context(tc.tile_pool(name="m1p", bufs=5))
        mSp = ctx.enter_context(tc.tile_pool(name="mSp", bufs=5))
        scp = ctx.enter_context(tc.tile_pool(name="scp", bufs=9))
        accp = ctx.enter_context(tc.tile_pool(name="accp", bufs=4, space="PSUM"))
        accEp = ctx.enter_context(tc.tile_pool(name="accEp", bufs=4, space="PSUM"))
        outp = ctx.enter_context(tc.tile_pool(name="outp", bufs=3))
        smallp = ctx.enter_context(tc.tile_pool(name="smallp", bufs=6))

        whA = whW.ap()[0 : ZP + HALF, :].bitcast(f32)
        whB = whW.ap()[ZP + HALF : n_pad + 2 * ZP, :].bitcast(f32)
        sdTf = sdT.ap().bitcast(f32)

        cbA = cbB = cbN = 0
        for j in range(J):
            na, nb, nch = CPBA[j], CPBB[j], NCH[j]
            m0 = m0p.tile([P, cpbmax, OUT_DIM], bf16)
            m1 = m1p.tile([P, cpbmax, A32], f32)
            mS = mSp.tile([P, cpbmax, A32], f32)
            # dst-side gather first: [one-hot | a_dst] rows from sdT
            for c0 in range(0, nch, GMAX):
                cn = min(GMAX, nch - c0)
                nc.gpsimd.dma_gather(
                    out_ap=mS[:, c0 : c0 + cn, :],
                    in_ap=sdTf,
                    idxs_ap=gS_sb[:, (cbN + c0) * 8 : (cbN + c0 + cn) * 8],
                    num_idxs=cn * P, num_idxs_reg=cn * P,
                    elem_size=A32, elem_step=A32,
                )
            # src gathers: a_src rows then Wh rows, A then B segments
            for tab, taba, nseg, cb, gsb, off in (
                (whA, auxA, na, cbA, gA_sb, 0),
                (whB, auxB, nb, cbB, gB_sb, na),
            ):
                for c0 in range(0, nseg, GMAX):
                    cn = min(GMAX, nseg - c0)
                    idxs = gsb[:, (cb + c0) * 8 : (cb + c0 + cn) * 8]
                    nc.gpsimd.dma_gather(
                        out_ap=m1[:, off + c0 : off + c0 + cn, :],
                        in_ap=taba, idxs_ap=idxs,
                        num_idxs=cn * P, num_idxs_reg=cn * P,
                        elem_size=A32, elem_step=A32,
                    )
                    nc.gpsimd.dma_gather(
                        out_ap=m0[:, off + c0 : off + c0 + cn, :].bitcast(f32),
                        in_ap=tab, idxs_ap=idxs,
                        num_idxs=cn * P, num_idxs_reg=cn * P,
                        elem_size=W32, elem_step=W32,
                    )
            m1b = m1[:].bitcast(bf16)
            mSb = mS[:].bitcast(bf16)
            # e = exp(lrelu(a_src + a_dst)); pairs-duplicated bf16
            s_t = scp.tile([P, cpbmax, K], f32)
            nc.vector.tensor_tensor(
                out=s_t[:, 0:nch, :],
                in0=m1b[:, 0:nch, 0:K],
                in1=mSb[:, 0:nch, BP:AUXW],
                op=mybir.AluOpType.add)
            lk = scp.tile([P, cpbmax, K], f32)
            nc.vector.scalar_tensor_tensor(
                out=lk[:, 0:nch, :], in0=s_t[:, 0:nch, :], scalar=NEG_SLOPE,
                in1=s_t[:, 0:nch, :],
                op0=mybir.AluOpType.mult, op1=mybir.AluOpType.max,
            )
            e2 = scp.tile([P, cpbmax, 2 * K], bf16)
            e2v = e2[:]
            for half in range(2):
                nc.scalar.activation(
                    out=bass.AP(tensor=e2v.tensor, offset=e2v.offset + half,
                                ap=[e2v.ap[0], [2 * K, nch], [2, K]]),
                    in_=lk[:, 0:nch, :],
                    func=mybir.ActivationFunctionType.Exp)
            # msgs *= bcast(e); (chunk,k)-flattened, last dims [1,2] -> 2x
            m0v = m0[:]
            nc.vector.tensor_tensor(
                out=bass.AP(tensor=m0v.tensor, offset=m0v.offset,
                            ap=[m0v.ap[0], [DK, nch * K],
                                [2, DK // 2], [1, 2]]),
                in0=bass.AP(tensor=m0v.tensor, offset=m0v.offset,
                            ap=[m0v.ap[0], [DK, nch * K],
                                [2, DK // 2], [1, 2]]),
                in1=bass.AP(tensor=e2v.tensor, offset=e2v.offset,
                            ap=[e2v.ap[0], [2, nch * K],
                                [0, DK // 2], [1, 2]]),
                op=mybir.AluOpType.mult,
            )
            # accumulate: acc[0:120] += sel^T msgs ; accE[0:120] += sel^T e2
            acc = accp.tile([P, OUT_DIM], f32)
            accE = accEp.tile([P, 2 * K], f32)
            for ci in range(nch):
                selci = mSb[:, ci, 0:BP]
                nc.tensor.matmul(
                    out=acc[0:BP, :],
                    lhsT=selci,
                    rhs=m0[:, ci, :],
                    start=(ci == 0),
                    stop=(ci == nch - 1),
                )
                nc.tensor.matmul(
                    out=accE[0:BP, :],
                    lhsT=selci,
                    rhs=e2[:, ci, :],
                    start=(ci == 0),
                    stop=(ci == nch - 1),
                )
            accEv = accE[:]
            r = smallp.tile([P, K], f32)
            nc.scalar.activation(
                out=r[0:BP, :],
                in_=bass.AP(tensor=accEv.tensor, offset=accEv.offset,
                            ap=[[accEv.ap[0][0], BP], [2, K]]),
                func=mybir.ActivationFunctionType.Copy, bias=1e-38,
            )
            nc.vector.reciprocal(out=r[0:BP, :], in_=r[0:BP, :])
            ot = outp.tile([P, OUT_DIM], f32)
            nc.vector.tensor_tensor(
                out=ot[0:BP, :], in0=acc[0:BP, :],
                in1=r[0:BP, :].to_broadcast([BP, K, DK]),
                op=mybir.AluOpType.mult,
            )
            nc.sync.dma_start(out=out_d.ap()[j * BP : (j + 1) * BP, :],
                              in_=ot[0:BP, :])
            cbA += na
            cbB += nb
            cbN += nch

    nc.compile()
    return nc


def run(h, edge_src, edge_dst, W, attn, n_cores=N_CORES, trace=False):
    import ml_dtypes
    from concourse.bass_utils import run_bass_kernel_spmd

    bf = ml_dtypes.bfloat16
    n_nodes = h.shape[0]
    h = np.asarray(h, dtype=np.float32)
    W = np.asarray(W, dtype=np.float32)
    attn = np.asarray(attn, dtype=np.float32)
    edge_src = np.asarray(edge_src)
    edge_dst = np.asarray(edge_dst)

    plan = build_plan(edge_src, edge_dst, n_nodes, n_cores)
    n_pad = plan["n_pad"]
    J = plan["J"]
    hTd = np.zeros((IN_DIM, n_pad), dtype=bf)
    hTd[:, :n_nodes] = h.T.astype(bf)
    c = (attn[:, :, None] * W.reshape(K, DK, IN_DIM)).sum(axis=1)
    waugT = np.concatenate([W.T, c.T], axis=1).astype(bf)
    # static one-hot rows for sdT: row j*BP+d = [onehot_120(d) | zeros(8)]
    soh1 = np.zeros((BP, 128), dtype=np.float32)
    soh1[:, :BP] = np.eye(BP, dtype=np.float32)
    soh = np.tile(soh1.astype(bf), (J, 1))

    nc = build_program(plan, n_cores)

    in_maps = []
    for cix in range(n_cores):
        in_maps.append({
            "hT": hTd,
            "waugT": waugT,
            "gA": plan["gA"][cix],
            "gB": plan["gB"][cix],
            "gS": plan["gS"][cix],
            "gRA": plan["gRA"][cix],
            "gRB": plan["gRB"][cix],
            "soh": soh,
        })
    try:
        res = run_bass_kernel_spmd(nc, in_maps, list(range(n_cores)), trace=trace)
    except Exception:
        if not trace:
            raise
        res = run_bass_kernel_spmd(nc, in_maps, list(range(n_cores)), trace=False)

    out_full = np.zeros((plan["B"] * BP, OUT_DIM), dtype=np.float32)
    for cix in range(n_cores):
        o = res.results[cix]["out"]
        for j in range(plan["J"]):
            b = plan["assign"][cix, j]
            if b >= 0:
                out_full[b * BP : (b + 1) * BP] = o[j * BP : (j + 1) * BP]
    out = out_full[:n_nodes].reshape(n_nodes, K, DK)
    return out, res


def kernel(h, edge_src, edge_dst, W, attn):
    out, _ = run(h, edge_src, edge_dst, W, attn)
    return out
